# revision 58
# baseline (speedup 1.0000x reference)
"""Causal single-head attention on 8 trn2 NeuronCores.

Problem (hardcoded): x [256,256,384] f32, Wq/Wk/Wv [384,64] f32
  q,k,v = x@W;  S = q@k^T * 384**-0.5; causal softmax; out = P@v  [256,256,64]

Sharding: data-parallel over batch B=256 -> 32 batches per core; weights
replicated.  All PE math in bf16 (inputs host-cast), fp32 PSUM accumulate.

Per core (NB=32 batches, DMA-grouped by G=8):

  1. x^T lands in SBUF bf16 via DMA-transpose straight from DRAM (one
     instruction per 128-channel chunk per group; group 0 split [2,6] so
     batch 0's data lands ~4us earlier).  No PE transposes, no cast, no
     PSUM evacuation for x.
  2. Stacked projection [Wk|Wq]: one matmul chain -> kq^T [128,256] PSUM
     (rows 0:64 k^T, 64:128 q^T).  v in natural [token, 64] layout lands in
     the same PSUM tile; kq evacuates on DVE, v on ACT.
  3. q^T moved to partitions 0:64 with a partition-shifted gpsimd copy
     (SBUF->SBUF) out of the kq tile.
  4. S^T as [self0|self1|cross] (fully-masked block skipped: 384 cols);
     the causal mask is a -30000 additive bias pre-filled into the two
     self-blocks by ONE identity matmul issued a stage early (the batch's
     PSUM bank is idle then), so a single ACT exp [128,384] produces the
     masked P^T with no separate mask op.  Accumulation groups stay
     contiguous per PSUM bank: start=True clears has_written bank-wide
     on real HW.
  5. O computed NATURALLY (lhsT = P^T chunks, rhs = v chunks); softmax
     denominators via 3 N=1 matmuls against a static ones column, issued
     BEFORE the O matmuls so the reciprocal overlaps them.  PSUM matmul
     accumulation groups are kept contiguous per bank (start=True clears
     has_written bank-wide on real HW).
     Per-partition reciprocal + single broadcast multiply -> bf16 out tile.
  6. One t-major store per group ([128, G*128] contiguous); the last
     group drains in staggered sub-stores ([4,2,1,1] batches) so the
     final store covers only batch 31 and the tail shrinks.  Host undoes
     the layout and upcasts to f32.
"""
import numpy as np

N_CORES = 8
B, T, C, H = 256, 256, 384, 64
NB = B // N_CORES          # 32 batches per core
G = 8                      # batches per DMA group
NG = NB // G               # 4 groups
SCALE = float(C) ** -0.5

_state = {}


def _build():
    import concourse.bacc as bacc
    import concourse.tile as tile
    import concourse.mybir as mybir
    from concourse.bass import AP
    from concourse.masks import make_identity

    dt = mybir.dt
    f32 = dt.float32
    bf16 = dt.bfloat16
    AF = mybir.ActivationFunctionType

    nc = bacc.Bacc("TRN2", target_bir_lowering=False)
    x_d = nc.dram_tensor("x", [NB, T, C], bf16, kind="ExternalInput")
    w_d = nc.dram_tensor("W", [128, 576], bf16, kind="ExternalInput")
    out_d = nc.dram_tensor("out", [128, NB * 128], bf16, kind="ExternalOutput")

    with tile.TileContext(nc) as tc:
        with tc.tile_pool(name="setup", bufs=1) as setup, \
             tc.tile_pool(name="xin", bufs=1) as xin, \
             tc.tile_pool(name="og", bufs=3) as ogp, \
             tc.tile_pool(name="pa", bufs=3, space="PSUM") as pa, \
             tc.tile_pool(name="pb", bufs=3, space="PSUM") as pb, \
             tc.tile_pool(name="po", bufs=2, space="PSUM") as po, \
             tc.tile_pool(name="kvw", bufs=32) as kvw, \
             tc.tile_pool(name="qw", bufs=32) as qw, \
             tc.tile_pool(name="ptw", bufs=32) as ptw, \
             tc.tile_pool(name="rw", bufs=32) as rw:

            # --- one-time setup ---
            ident = setup.tile([128, 128], bf16)
            make_identity(nc, ident)
            bmask = setup.tile([128, 128], bf16)
            nc.gpsimd.memset(bmask, 0.0)
            nc.gpsimd.affine_select(
                out=bmask, in_=bmask,
                compare_op=mybir.AluOpType.is_ge,
                fill=-30000.0, base=0,
                pattern=[[1, 128]], channel_multiplier=-1)
            bmask2 = setup.tile([128, 256], bf16)
            nc.vector.tensor_copy(bmask2[:, 0:128], bmask)
            nc.vector.tensor_copy(bmask2[:, 128:256], bmask)
            ones1 = setup.tile([128, 1], bf16)
            nc.vector.memset(ones1, 1.0)
            w_s = setup.tile([128, 576], bf16)
            nc.sync.dma_start(out=w_s, in_=w_d[:, :])
            wkq_s = w_s[:, 0:384]
            wv_s = w_s[:, 384:576]

            # PE warm-up during initial DMA latency (p-state ramp is ~3us)
            warm = setup.tile([128, 128], bf16)
            nc.gpsimd.memset(warm, 0.0)
            wps = pa.tile([128, 384], f32, tag="pat")
            for _ in range(40):
                nc.tensor.matmul(wps[:, 0:128], warm, warm,
                                 start=True, stop=True)

            xts = []

            def load_group(g, nsplit=1, subs=None):
                ts = [xin.tile([128, G * T], bf16, tag=f"xt{cc}", bufs=3,
                               name=f"xt{cc}") for cc in range(3)]
                if subs is None:
                    bs = G // nsplit
                    subs = [bs] * nsplit
                off = 0
                for bs in subs:
                    for cc in range(3):
                        b0 = g * G + off
                        src = x_d[b0:b0 + bs, :, cc * 128:(cc + 1) * 128]
                        src = src.rearrange("g t c -> (g t) c")
                        nc.sync.dma_start(
                            out=ts[cc][:, off * T:(off + bs) * T],
                            in_=src, transpose=True)
                    off += bs
                return ts

            xts.append(load_group(0, subs=[2, 6]))
            xts.append(load_group(1, nsplit=2))

            # --- software-pipelined batch loop (stages skewed by batch
            # so every engine's in-order stream is dependency-ready) ---
            st_ = {}   # per-batch tile dict

            def stage_front(b):
                g, i = divmod(b, G)
                xtg = xts[g]
                xcol = i * T
                pat = pa.tile([128, 384], f32, tag="pat", name="pat")
                kv_s = kvw.tile([128, 384], bf16, tag="kv", name="kv_s")
                st_[b] = {"pat": pat, "kv": kv_s, "xtg": xtg, "xcol": xcol}
                kqps = pat[:, 0:256]
                vps = pat[:, 256:384]
                for cc in range(3):
                    nc.tensor.matmul(kqps,
                                     wkq_s[:, cc * 128:(cc + 1) * 128],
                                     xtg[cc][:, xcol:xcol + 256],
                                     start=(cc == 0), stop=(cc == 2))
                for tcx in range(2):
                    for cc in range(3):
                        nc.tensor.matmul(
                            vps[:, tcx * 64:(tcx + 1) * 64],
                            xtg[cc][:, xcol + tcx * 128:xcol + (tcx + 1) * 128],
                            wv_s[:, cc * 64:(cc + 1) * 64],
                            start=(cc == 0), stop=(cc == 2))
                nc.vector.tensor_copy(kv_s[:, 0:256], kqps)
                nc.scalar.copy(kv_s[:, 256:384], vps)
                stps = pb.tile([128, 384], f32, tag="stps", name="stps")
                st_[b]["stps"] = stps
                nc.tensor.matmul(stps[:, 0:256], ident, bmask2,
                                 start=True, stop=False)

            def stage_mid(b):
                s = st_[b]
                kv_s = s["kv"]
                q_s = qw.tile([64, 256], bf16, tag="qs", name="q_s")
                stps = s["stps"]
                pt = ptw.tile([128, 384], bf16, tag="pt", name="pt")
                s.update(qs=q_s, pt=pt)
                nc.gpsimd.tensor_copy(q_s[0:64, :], kv_s[64:128, 0:256])
                nc.tensor.matmul(stps[:, 0:128], kv_s[0:64, 0:128],
                                 q_s[:, 0:128], start=False, stop=True,
                                 skip_group_check=True)
                nc.tensor.matmul(stps[:, 128:256], kv_s[0:64, 128:256],
                                 q_s[:, 128:256], start=False, stop=True,
                                 skip_group_check=True)
                nc.tensor.matmul(stps[:, 256:384], kv_s[0:64, 0:128],
                                 q_s[:, 128:256], start=True, stop=True)
                nc.scalar.activation(pt, stps, AF.Exp, scale=SCALE)

            def stage_back(b):
                g, i = divmod(b, G)
                s = st_[b]
                kv_s, pt = s["kv"], s["pt"]
                pot = po.tile([128, 130], f32, tag="pot", name="pot")
                rec = rw.tile([128, 2], f32, tag="rec", name="rec")
                og = ogs[g]
                # NOTE: start=True clears has_written bank-wide, so each
                # accumulation group must run contiguously on this bank.
                nc.tensor.matmul(pot[:, 128:129], pt[:, 0:128], ones1,
                                 start=True, stop=True)
                nc.tensor.matmul(pot[:, 129:130], pt[:, 256:384], ones1,
                                 start=True, stop=False)
                nc.tensor.matmul(pot[:, 129:130], pt[:, 128:256], ones1,
                                 start=False, stop=True)
                nc.tensor.matmul(pot[:, 0:64], pt[:, 0:128],
                                 kv_s[:, 256:320], start=True, stop=True)
                nc.tensor.matmul(pot[:, 64:128], pt[:, 256:384],
                                 kv_s[:, 256:320], start=True, stop=False)
                nc.tensor.matmul(pot[:, 64:128], pt[:, 128:256],
                                 kv_s[:, 320:384], start=False, stop=True)
                nc.vector.reciprocal(rec, pot[:, 128:130])
                srcv = AP(pot.tensor, pot.offset,
                          [pot.ap[0], [64, 2], [1, 64]])
                rbc = AP(rec.tensor, rec.offset,
                         [rec.ap[0], [1, 2], [0, 64]])
                dst = og[:, i * 128:(i + 1) * 128]
                dst = AP(dst.tensor, dst.offset,
                         [dst.ap[0], [64, 2], [1, 64]])
                nc.vector.tensor_mul(dst, srcv, rbc)
                del st_[b]

            ogs = {}
            for ii in range(NB + 2):
                if ii < NB:
                    g = ii // G
                    if g not in ogs:
                        ogs[g] = ogp.tile([128, G * 128], bf16, tag="og",
                                          name="og")
                    stage_front(ii)
                if 1 <= ii <= NB:
                    stage_mid(ii - 1)
                if ii >= 2:
                    b2 = ii - 2
                    stage_back(b2)
                    g2, i2 = divmod(b2, G)
                    if g2 == NG - 1 and i2 == G // 2 - 1:
                        nc.sync.dma_start(
                            out=out_d[:, g2 * G * 128:g2 * G * 128 + G * 64],
                            in_=ogs[g2][:, 0:G * 64])
                    if g2 == NG - 1 and i2 == G - 3:
                        nc.sync.dma_start(
                            out=out_d[:, g2 * G * 128 + G * 64:
                                      g2 * G * 128 + G * 64 + 2 * 128],
                            in_=ogs[g2][:, G * 64:G * 64 + 2 * 128])
                    if g2 == NG - 1 and i2 == G - 2:
                        nc.sync.dma_start(
                            out=out_d[:, g2 * G * 128 + G * 64 + 2 * 128:
                                      g2 * G * 128 + G * 64 + 3 * 128],
                            in_=ogs[g2][:, G * 64 + 2 * 128:G * 64 + 3 * 128])
                    if i2 == G - 1:  # group complete -> store + prefetch
                        if g2 == NG - 1:
                            nc.sync.dma_start(
                                out=out_d[:, (g2 + 1) * G * 128 - 128:
                                          (g2 + 1) * G * 128],
                                in_=ogs[g2][:, G * 128 - 128:G * 128])
                        else:
                            nc.sync.dma_start(
                                out=out_d[:, g2 * G * 128:(g2 + 1) * G * 128],
                                in_=ogs[g2])
                        if g2 + 2 < NG:
                            xts.append(load_group(g2 + 2, nsplit=2))

    nc.finalize()
    return nc


def kernel(x, Wq, Wk, Wv, _trace=False):
    import ml_dtypes
    from concourse.bass_utils import run_bass_kernel_spmd

    if "nc" not in _state:
        _state["nc"] = _build()
    nc = _state["nc"]

    bf16 = ml_dtypes.bfloat16
    x = np.ascontiguousarray(np.asarray(x, dtype=np.float32)).astype(bf16)
    wkq = np.concatenate(
        [np.asarray(Wk, np.float32), np.asarray(Wq, np.float32)], axis=1)
    wkq = wkq.reshape(3, 128, 128).transpose(1, 0, 2).reshape(128, 384)
    wv = np.asarray(Wv, np.float32).reshape(3, 128, 64)
    wv = wv.transpose(1, 0, 2).reshape(128, 192)
    w = np.ascontiguousarray(
        np.concatenate([wkq, wv], axis=1)).astype(bf16)

    in_maps = [
        {"x": x[i * NB:(i + 1) * NB], "W": w}
        for i in range(N_CORES)
    ]
    res = run_bass_kernel_spmd(nc, in_maps, core_ids=list(range(N_CORES)),
                               trace=_trace)
    _state["exec_time_ns"] = res.exec_time_ns
    _state["trace"] = res.instructions_and_trace

    outs = []
    for i in range(N_CORES):
        o = np.asarray(res.results[i]["out"]).astype(np.float32)
        # [128, NB*128] t-major -> [NB, 256, 64]
        o = o.reshape(128, NB, 2, 64).transpose(1, 2, 0, 3).reshape(NB, T, H)
        outs.append(o)
    return np.concatenate(outs, axis=0)
